# revision 5
# baseline (speedup 1.0000x reference)
"""Trainium2 Bass kernel for DeepGraphTransformer (gnn_message_passing).

Sharding (8 NeuronCores, SPMD):
  - Nodes padded 10000->10240; core c owns dst nodes [c*1280, (c+1)*1280)
    (10 blocks of 128). Edges sorted by dst, sharded by dst range, chunked
    128 per dst-block (block-aligned, padded with masked edges; the per-block
    chunk count is made uniform across cores so one program serves all).
  - h^T (bf16, [128c, 10240]) replicated via a per-layer AllGather; k/v
    projections computed redundantly on every core, staged in HBM, and
    edge-gathered with dma_gather (k transposed for the PE; v in row layout
    with a fused ones column that yields the softmax denominator for free).
  - Scores per chunk: S = ks^T . q_blk on the PE; exp on ACT (segment-max
    subtraction dropped: softmax ratios are invariant and logits are O(1));
    W = exp(S) * onehot on DVE; aggregation [sum_e W v | sum_e W] in one PE
    matmul accumulated in PSUM per dst block. Normalization, beta gating,
    LayerNorms and the FFN run on the core's own 1280 nodes.
  - Attentional pooling via per-graph one-hot matmul partials + a 33KB
    AllReduce; the output MLP is computed redundantly; core 0's OUT is used.
"""

import numpy as np
import ml_dtypes

import concourse.bacc as bacc
import concourse.mybir as mybir
import concourse.tile as tile
from concourse.bass_utils import run_bass_kernel_spmd
from concourse.masks import make_identity

P = 128
N = 10000
NPAD = 10240
HID = 128
HEADS = 8
NCORES = 8
PB = NPAD // NCORES          # nodes per core (1280)
NBLK = PB // P               # dst blocks per core (10)
NT = NPAD // P               # node tiles (80)
NG = 64                      # graphs
LAYERS = 6
SUP = 4                      # chunks per superchunk (gather granularity)
VROW = HEADS * 144           # v_hbm row: per head 128 v + 1 one + 15 pad
f32 = mybir.dt.float32
bf16 = mybir.dt.bfloat16
i16 = mybir.dt.int16
AF = mybir.ActivationFunctionType
ALU = mybir.AluOpType
X = mybir.AxisListType.X
INV_SQRT_HID = float(1.0 / np.sqrt(HID))
S_EPS = 1e-12


def _bf(x):
    return np.ascontiguousarray(np.asarray(x, np.float32)).astype(ml_dtypes.bfloat16)


def _f32(x):
    return np.ascontiguousarray(np.asarray(x, np.float32))


def _wrap16(idx_flat, nchunk):
    """Edge-order idx list -> [128, nchunk*8] int16 wrapped-16 table."""
    cols = SUP * 8
    w = np.zeros((P, nchunk * 8), np.int16)
    for sc in range(nchunk // SUP):
        seg = idx_flat[sc * SUP * P:(sc + 1) * SUP * P].reshape(cols, 16).T
        w[:, sc * cols:(sc + 1) * cols] = np.tile(seg, (8, 1))
    return w


def _preprocess(x, edge_index, batch, params):
    """Host-side static preprocessing -> per-core input maps + chunk layout."""
    src = np.asarray(edge_index[0], np.int64)
    dst = np.asarray(edge_index[1], np.int64)
    order = np.argsort(dst, kind="stable")
    s_s, d_s = src[order], dst[order]

    PADCH = (np.full(P, N, np.int64), np.zeros(P, np.int64), np.zeros(P, bool))
    by_blk_core = [[[] for _ in range(NBLK)] for _ in range(NCORES)]
    for c in range(NCORES):
        lo = c * PB
        sel = (d_s >= lo) & (d_s < lo + PB)
        cs, cd = s_s[sel], d_s[sel]
        for b in range(NBLK):
            bsel = (cd >= lo + b * P) & (cd < lo + (b + 1) * P)
            es, ed = cs[bsel], cd[bsel] - (lo + b * P)
            n = len(es)
            nch = max(1, (n + P - 1) // P)
            pad = nch * P - n
            es = np.concatenate([es, np.full(pad, N, np.int64)])
            ed = np.concatenate([ed, np.zeros(pad, np.int64)])
            real = np.concatenate([np.ones(n, bool), np.zeros(pad, bool)])
            for j in range(nch):
                sl = slice(j * P, (j + 1) * P)
                by_blk_core[c][b].append((es[sl], ed[sl], real[sl]))

    # uniform per-block chunk counts across cores (SPMD: one block schedule)
    blk_nch = [max(len(by_blk_core[c][b]) for c in range(NCORES))
               for b in range(NBLK)]
    nchunk = sum(blk_nch)
    extra = ((nchunk + SUP - 1) // SUP) * SUP - nchunk
    blk_nch[-1] += extra
    nchunk += extra
    blk_sched = []
    for b in range(NBLK):
        blk_sched += [b] * blk_nch[b]
    core_chunks = []
    for c in range(NCORES):
        chunks = []
        for b in range(NBLK):
            lst = by_blk_core[c][b]
            lst = lst + [PADCH] * (blk_nch[b] - len(lst))
            chunks.extend(lst)
        core_chunks.append(chunks)

    srcw, oneh, ohg, padm, xown = [], [], [], [], []
    xp = np.zeros((NPAD, 4), np.float32)
    xp[:N] = _f32(x)
    bat = np.asarray(batch, np.int64)
    nodes = np.arange(NPAD)
    for c in range(NCORES):
        idx_flat = np.concatenate([ch[0] for ch in core_chunks[c]])
        srcw.append(_wrap16(idx_flat.astype(np.int16), nchunk))
        oh = np.zeros((P, nchunk, P), np.float32)
        for j, (_es, ed, real) in enumerate(core_chunks[c]):
            oh[np.arange(P)[real], j, ed[real]] = 1.0
        oneh.append(_bf(oh))
        own = nodes[c * PB:(c + 1) * PB]
        og = np.zeros((PB, NG), np.float32)
        valid = own < N
        og[np.arange(PB)[valid], bat[own[valid]]] = 1.0
        ohg.append(_bf(og.reshape(NBLK, P, NG).transpose(1, 0, 2)))
        padm.append(_f32(valid.astype(np.float32).reshape(NBLK, P).T))
        xo = np.zeros((PB, 4), np.float32)
        n1 = min((c + 1) * PB, N)
        if n1 > c * PB:
            xo[:n1 - c * PB] = _f32(x)[c * PB:n1]
        xown.append(xo)

    rep = lambda v: np.tile(_f32(v).reshape(1, -1), (P, 1))  # noqa: E731
    w = {}
    w["Win"] = _f32(params["Win"])
    w["binc"] = _f32(params["bin"]).reshape(P, 1)
    w["binb"] = rep(params["bin"])
    for li, p in enumerate(params["layers"]):
        w[f"L{li}_Wq"] = _bf(_f32(p["Wq"]).reshape(HID, HEADS, HID))
        w[f"L{li}_bq"] = _f32(p["bq"]).reshape(HEADS, HID).T
        w[f"L{li}_Wk"] = _bf(p["Wk"])
        Wvp = np.zeros((HID, HEADS, 144), np.float32)
        Wvp[:, :, :HID] = _f32(p["Wv"]).reshape(HID, HEADS, HID)
        w[f"L{li}_Wv"] = _bf(Wvp.reshape(HID, VROW))
        w[f"L{li}_bvb"] = rep(_f32(p["bv"]).reshape(HEADS, HID).mean(0))
        w[f"L{li}_Wskip"] = _bf(p["Wskip"])
        w[f"L{li}_bskipb"] = rep(p["bskip"])
        Wb = _f32(p["Wbeta"]).reshape(3 * HID)
        w[f"L{li}_wab"] = rep(Wb[:HID] + Wb[2 * HID:])
        w[f"L{li}_wbb"] = rep(Wb[HID:2 * HID] - Wb[2 * HID:])
        w[f"L{li}_g1b"] = rep(p["g1"])
        w[f"L{li}_b1b"] = rep(p["n1"])
        w[f"L{li}_g2b"] = rep(p["g2"])
        w[f"L{li}_b2b"] = rep(p["n2"])
        w[f"L{li}_W1"] = _bf(p["W1"])
        w[f"L{li}_bf1b"] = rep(p["bf1"])
        W2 = _f32(p["W2"])
        w[f"L{li}_W2a"] = _bf(W2[:HID])
        w[f"L{li}_W2b"] = _bf(W2[HID:])
        w[f"L{li}_bf2b"] = rep(p["bf2"])
    w["Wg1"] = _bf(params["Wg1"])
    w["bg1b"] = rep(params["bg1"])
    w["wg2b"] = rep(_f32(params["Wg2"])[:, 0])
    w["Wo1"] = _bf(params["Wo1"])
    w["bo1c"] = _f32(params["bo1"]).reshape(P, 1)
    w["Wo2"] = _bf(params["Wo2"])
    w["bo2c"] = _f32(params["bo2"]).reshape(2, 1)
    bg2 = float(np.asarray(params["bg2"]).reshape(-1)[0])

    in_maps = []
    for c in range(NCORES):
        m = dict(w)
        m["xpad"] = xp
        m["xown"] = xown[c]
        m["src16"] = srcw[c]
        m["onehot"] = oneh[c]
        m["ohg"] = ohg[c]
        m["padm"] = padm[c]
        in_maps.append(m)
    return in_maps, blk_sched, nchunk, bg2


def _ln(nc, sm, out, x, gb, bb):
    """LayerNorm along the free axis of a [128, 128] tile."""
    mu = sm.tile([P, 1], f32, tag="ln_mu", name="ln_mu")
    nc.vector.reduce_sum(mu[:], x[:], axis=X)
    nc.vector.tensor_scalar_mul(mu[:], mu[:], 1.0 / HID)
    xm = sm.tile([P, P], f32, tag="ln_xm", name="ln_xm")
    nc.vector.tensor_scalar(xm[:], x[:], mu[:, :1], None, ALU.subtract)
    sq = sm.tile([P, P], f32, tag="ln_sq", name="ln_sq")
    var = sm.tile([P, 1], f32, tag="ln_var", name="ln_var")
    nc.scalar.activation(sq[:], xm[:], AF.Square, accum_out=var[:, :1])
    nc.vector.tensor_scalar(var[:], var[:], 1.0 / HID, 1e-5, ALU.mult, ALU.add)
    sd = sm.tile([P, 1], f32, tag="ln_sd", name="ln_sd")
    nc.scalar.sqrt(sd[:], var[:])
    nc.vector.reciprocal(sd[:], sd[:])
    nc.vector.tensor_scalar(xm[:], xm[:], sd[:, :1], None, ALU.mult)
    nc.vector.tensor_tensor(xm[:], xm[:], gb[:], ALU.mult)
    nc.vector.tensor_add(out[:], xm[:], bb[:])


def _build(blk_sched, nchunk, bg2, layers=LAYERS, debug=False):
    nc = bacc.Bacc()
    ti = {}

    def inp(name, shape, dt):
        ti[name] = nc.dram_tensor(name, list(shape), dt, kind="ExternalInput")

    inp("xpad", (NPAD, 4), f32)
    inp("xown", (PB, 4), f32)
    inp("src16", (P, nchunk * 8), i16)
    inp("onehot", (P, nchunk, P), bf16)
    inp("ohg", (P, NBLK, NG), bf16)
    inp("padm", (P, NBLK), f32)
    inp("Win", (4, P), f32)
    inp("binc", (P, 1), f32)
    inp("binb", (P, P), f32)
    for li in range(layers):
        inp(f"L{li}_Wq", (P, HEADS, P), bf16)
        inp(f"L{li}_bq", (P, HEADS), f32)
        inp(f"L{li}_Wk", (P, HEADS * HID), bf16)
        inp(f"L{li}_Wv", (P, VROW), bf16)
        for nm in ("bvb", "bskipb", "wab", "wbb", "g1b", "b1b", "g2b", "b2b",
                   "bf2b"):
            inp(f"L{li}_{nm}", (P, P), f32)
        inp(f"L{li}_Wskip", (P, P), bf16)
        inp(f"L{li}_W1", (P, 2 * HID), bf16)
        inp(f"L{li}_bf1b", (P, 2 * HID), f32)
        inp(f"L{li}_W2a", (P, P), bf16)
        inp(f"L{li}_W2b", (P, P), bf16)
    inp("Wg1", (P, P), bf16)
    inp("bg1b", (P, P), f32)
    inp("wg2b", (P, P), f32)
    inp("Wo1", (P, P), bf16)
    inp("bo1c", (P, 1), f32)
    inp("Wo2", (P, 2), bf16)
    inp("bo2c", (2, 1), f32)
    OUT = nc.dram_tensor("OUT", [NG, 2], f32, kind="ExternalOutput")
    if debug:
        DBG_H = nc.dram_tensor("DBG_H", [PB, P], f32, kind="ExternalOutput")
        DBG_ATT = nc.dram_tensor("DBG_ATT", [PB, P], f32, kind="ExternalOutput")

    NSUP = nchunk // SUP
    blk_first, blk_last = [None] * NBLK, [None] * NBLK
    for j, b in enumerate(blk_sched):
        if blk_first[b] is None:
            blk_first[b] = j
        blk_last[b] = j

    with tile.TileContext(nc) as tc:
        with (
            tc.tile_pool(name="persist", bufs=1) as pp,
            tc.tile_pool(name="dram", bufs=1, space="DRAM") as dr,
            tc.tile_pool(name="wpool", bufs=2) as wp,
            tc.tile_pool(name="stage", bufs=3) as stg,
            tc.tile_pool(name="gat", bufs=2) as gp,
            tc.tile_pool(name="small", bufs=4) as sm,
            tc.tile_pool(name="edge", bufs=3) as ep,
            tc.tile_pool(name="psA", bufs=2, space="PSUM") as psA,
            tc.tile_pool(name="psP", bufs=4, space="PSUM") as psP,
        ):
            big = lambda nm: psA.tile([P, 1024], f32, space="PSUM", tag="big",  # noqa: E731
                                      name=nm)
            bigb = lambda nm: psA.tile([P, 1024], bf16, space="PSUM", tag="big",  # noqa: E731
                                       name=nm)
            pair = lambda nm: psP.tile([P, 2, 132], f32, space="PSUM",  # noqa: E731
                                       tag="pair", name=nm)

            k_hbm = dr.tile([NPAD, HEADS * HID], bf16)
            v_hbm = dr.tile([NPAD, VROW], bf16)
            ag_in = dr.tile([PB, P], bf16)
            ar_in = dr.tile([NG, 132], f32)
            ar_out = dr.tile([NG, 132], f32, addr_space="Shared")

            # ---------------- persistent tiles ----------------
            hT = pp.tile([P, NPAD], bf16)
            hTo = pp.tile([P, PB], bf16)
            h_own = pp.tile([P, NBLK, P], f32)
            qT = pp.tile([P, HEADS, PB], bf16)
            src16_t = pp.tile([P, nchunk * 8], i16)
            ident = pp.tile([P, P], f32)
            ident_bf = pp.tile([P, P], bf16)
            ones8 = pp.tile([P, HEADS], bf16)
            padm_t = pp.tile([P, NBLK], f32)
            ohg_t = pp.tile([P, NBLK, NG], bf16)
            make_identity(nc, ident[:])
            nc.vector.tensor_copy(ident_bf[:], ident[:])
            nc.gpsimd.memset(ones8[:], 1.0)
            nc.sync.dma_start(src16_t[:], ti["src16"][:])
            nc.sync.dma_start(padm_t[:], ti["padm"][:])
            nc.sync.dma_start(ohg_t[:], ti["ohg"][:])

            # ---------------- input stage: h0 ----------------
            win_t = sm.tile([4, P], f32, tag="win")
            binc_t = sm.tile([P, 1], f32, tag="binc")
            binb_t = sm.tile([P, P], f32, tag="binb")
            nc.sync.dma_start(win_t[:], ti["Win"][:])
            nc.sync.dma_start(binc_t[:], ti["binc"][:])
            nc.sync.dma_start(binb_t[:], ti["binb"][:])
            for t in range(NT):
                xt = stg.tile([P, 4], f32, tag="xin", name="xt")
                nc.sync.dma_start(xt[:], ti["xpad"][t * P:(t + 1) * P, :])
                pxt = big("pxt")
                nc.tensor.transpose(pxt[:4, :P], xt[:], ident[:])
                xTt = stg.tile([4, P], f32, tag="xT", name="xTt")
                nc.any.tensor_copy(xTt[:], pxt[:4, :P])
                ph = big("ph")
                nc.tensor.matmul(ph[:, :P], lhsT=win_t[:], rhs=xTt[:],
                                 start=True, stop=True)
                nc.scalar.activation(hT[:, t * P:(t + 1) * P], ph[:, :P],
                                     AF.Identity, bias=binc_t[:, :1])
            for t in range(NBLK):
                xt = stg.tile([P, 4], f32, tag="xin", name="xt2")
                nc.sync.dma_start(xt[:], ti["xown"][t * P:(t + 1) * P, :])
                pxt = big("pxt2")
                nc.tensor.transpose(pxt[:4, :P], xt[:], ident[:])
                xTt = stg.tile([4, P], f32, tag="xT", name="xTt2")
                nc.any.tensor_copy(xTt[:], pxt[:4, :P])
                ph = big("ph2")
                nc.tensor.matmul(ph[:, :P], lhsT=xTt[:],
                                 rhs=win_t[:], start=True, stop=True)
                nc.vector.tensor_add(h_own[:, t, :], ph[:, :P], binb_t[:])
                hob = stg.tile([P, P], bf16, tag="hob", name="hob")
                nc.vector.tensor_copy(hob[:], h_own[:, t, :])
                ptr = bigb("ptr0")
                nc.tensor.transpose(ptr[:, :P], hob[:], ident_bf[:])
                nc.any.tensor_copy(hTo[:, t * P:(t + 1) * P], ptr[:, :P])

            # ---------------- layers ----------------
            for li in range(layers):
                LW = lambda nm: ti[f"L{li}_{nm}"]  # noqa: E731
                Wq_t = wp.tile([P, HEADS, P], bf16, tag="Wq", name="Wq_t")
                bq_t = wp.tile([P, HEADS], f32, tag="bq", name="bq_t")
                Wk_t = wp.tile([P, HEADS * HID], bf16, tag="Wk", name="Wk_t")
                Wv_t = wp.tile([P, VROW], bf16, tag="Wv", name="Wv_t")
                Wskip_t = wp.tile([P, P], bf16, tag="Wsk", name="Wskip_t")
                W1_t = wp.tile([P, 2 * HID], bf16, tag="W1", name="W1_t")
                W2a_t = wp.tile([P, P], bf16, tag="W2a", name="W2a_t")
                W2b_t = wp.tile([P, P], bf16, tag="W2b", name="W2b_t")
                for h_, nm in ((Wq_t, "Wq"), (bq_t, "bq"), (Wk_t, "Wk"),
                               (Wv_t, "Wv"), (Wskip_t, "Wskip"), (W1_t, "W1"),
                               (W2a_t, "W2a"), (W2b_t, "W2b")):
                    nc.sync.dma_start(h_[:], LW(nm)[:])
                cb = {}
                for nm in ("bvb", "bskipb", "wab", "wbb", "g1b", "b1b", "g2b",
                           "b2b", "bf2b"):
                    cb[nm] = wp.tile([P, P], f32, tag=nm, name=nm)
                    nc.sync.dma_start(cb[nm][:], LW(nm)[:])
                bf1b_t = wp.tile([P, 2 * HID], f32, tag="bf1b", name="bf1b_t")
                nc.sync.dma_start(bf1b_t[:], LW("bf1b")[:])

                # ---- phase A: k, v for all nodes -> HBM; own q^T ----
                for t in range(NT):
                    hTs = hT[:, t * P:(t + 1) * P]
                    pk = big("pk")
                    nc.tensor.matmul(pk[:, :512], lhsT=hTs, rhs=Wk_t[:, :512],
                                     start=True, stop=True)
                    nc.tensor.matmul(pk[:, 512:], lhsT=hTs, rhs=Wk_t[:, 512:],
                                     start=True, stop=True)
                    kst = stg.tile([P, 1024], bf16, tag="kst", name="kst")
                    nc.any.tensor_copy(kst[:], pk[:])
                    nc.sync.dma_start(k_hbm[t * P:(t + 1) * P, :], kst[:])
                    pv = big("pv")
                    nc.tensor.matmul(pv[:, :512], lhsT=hTs, rhs=Wv_t[:, :512],
                                     start=True, stop=True)
                    nc.tensor.matmul(pv[:, 512:], lhsT=hTs,
                                     rhs=Wv_t[:, 512:1024], start=True, stop=True)
                    pv2 = big("pv2")
                    nc.tensor.matmul(pv2[:, :VROW - 1024], lhsT=hTs,
                                     rhs=Wv_t[:, 1024:], start=True, stop=True)
                    vst = stg.tile([P, VROW], bf16, tag="vst", name="vst")
                    nc.any.tensor_copy(vst[:, :1024], pv[:])
                    nc.any.tensor_copy(vst[:, 1024:], pv2[:, :VROW - 1024])
                    nc.vector.tensor_copy(
                        vst[:].rearrange("p (h c) -> p h c", h=HEADS)[:, :, 128:129],
                        ones8[:, :, None])
                    nc.sync.dma_start(v_hbm[t * P:(t + 1) * P, :], vst[:])
                for t in range(NBLK):
                    pq = big("pq")
                    for h in range(HEADS):
                        nc.tensor.matmul(pq[:, h * P:(h + 1) * P],
                                         lhsT=Wq_t[:, h, :],
                                         rhs=hTo[:, t * P:(t + 1) * P],
                                         start=(h % 4 == 0), stop=(h % 4 == 3))
                    for h in range(HEADS):
                        nc.scalar.activation(qT[:, h, t * P:(t + 1) * P],
                                             pq[:, h * P:(h + 1) * P],
                                             AF.Identity, bias=bq_t[:, h:h + 1])

                # ---- phase B: edges ----
                ks_t, vs_t, oh_t = [None] * NSUP, [None] * NSUP, [None] * NSUP
                pairs = None
                for j in range(nchunk):
                    b = blk_sched[j]
                    sc, jj = divmod(j, SUP)
                    if jj == 0:
                        kst = gp.tile([P, HEADS, SUP * P], bf16, tag="ks", name="ksg")
                        nc.gpsimd.dma_gather(
                            out_ap=kst[:], in_ap=k_hbm[:],
                            idxs_ap=src16_t[:, sc * SUP * 8:(sc + 1) * SUP * 8],
                            num_idxs=SUP * P, num_idxs_reg=SUP * P,
                            elem_size=HEADS * HID, transpose=True)
                        vst = gp.tile([P, SUP, VROW], bf16, tag="vs", name="vsg")
                        nc.gpsimd.dma_gather(
                            out_ap=vst[:], in_ap=v_hbm[:],
                            idxs_ap=src16_t[:, sc * SUP * 8:(sc + 1) * SUP * 8],
                            num_idxs=SUP * P, num_idxs_reg=SUP * P,
                            elem_size=VROW)
                        oht = gp.tile([P, SUP, P], bf16, tag="oh", name="ohg2")
                        nc.sync.dma_start(
                            oht[:], ti["onehot"][:, sc * SUP:(sc + 1) * SUP, :])
                        ks_t[sc], vs_t[sc], oh_t[sc] = kst, vst, oht
                    kst, vst, oht = ks_t[sc], vs_t[sc], oh_t[sc]
                    if j == blk_first[b]:
                        pairs = [pair(f"pr{i}") for i in range(4)]
                    pS = big("pS")
                    for h in range(HEADS):
                        nc.tensor.matmul(
                            pS[:, h * P:(h + 1) * P],
                            lhsT=kst[:, h, jj * P:(jj + 1) * P],
                            rhs=qT[:, h, b * P:(b + 1) * P],
                            start=(h % 4 == 0), stop=(h % 4 == 3))
                    exS = ep.tile([P, HEADS, P], bf16, tag="exS", name="exS")
                    nc.scalar.activation(
                        exS[:].rearrange("p h c -> p (h c)"), pS[:],
                        AF.Exp, scale=INV_SQRT_HID)
                    Wt = ep.tile([P, HEADS, P], bf16, tag="Wt", name="Wt")
                    nc.vector.tensor_tensor(
                        Wt[:], exS[:],
                        oht[:, jj, None, :].to_broadcast([P, HEADS, P]),
                        ALU.mult)
                    first, last = j == blk_first[b], j == blk_last[b]
                    for h in range(HEADS):
                        nc.tensor.matmul(
                            pairs[h // 2][:, h % 2, :129],
                            lhsT=Wt[:, h, :],
                            rhs=vst[:, jj, h * 144:h * 144 + 129],
                            start=(first and h % 2 == 0),
                            stop=(last and h % 2 == 1))
                    if not last:
                        continue

                    # ---- block epilogue (own tile t = b) ----
                    t = b
                    att = ep.tile([P, P], f32, tag="att", name="att")
                    tmp = ep.tile([P, P], f32, tag="tmp", name="tmpe")
                    for h in range(HEADS):
                        uh = pairs[h // 2][:, h % 2, :128]
                        sh = pairs[h // 2][:, h % 2, 128:129]
                        rs = sm.tile([P, 1], f32, tag="rs", name="rs")
                        nc.vector.tensor_scalar(rs[:], sh, S_EPS, None, ALU.add)
                        nc.vector.reciprocal(rs[:], rs[:])
                        nc.vector.tensor_scalar_mul(rs[:], rs[:], 1.0 / HEADS)
                        if h == 0:
                            nc.vector.tensor_scalar(att[:], uh, rs[:, :1],
                                                    None, ALU.mult)
                        else:
                            nc.vector.tensor_scalar(tmp[:], uh, rs[:, :1],
                                                    None, ALU.mult)
                            nc.vector.tensor_add(att[:], att[:], tmp[:])
                    nc.vector.tensor_add(att[:], att[:], cb["bvb"][:])
                    if debug and li == 0:
                        nc.sync.dma_start(DBG_ATT[t * P:(t + 1) * P, :], att[:])
                    pxr = pair("pxr")
                    nc.tensor.matmul(pxr[:, 0, :P], lhsT=hTo[:, t * P:(t + 1) * P],
                                     rhs=Wskip_t[:], start=True, stop=True)
                    xr = ep.tile([P, P], f32, tag="xr", name="xr")
                    nc.vector.tensor_add(xr[:], pxr[:, 0, :P], cb["bskipb"][:])
                    m1 = ep.tile([P, P], f32, tag="m1", name="m1")
                    r1 = sm.tile([P, 1], f32, tag="r1", name="r1")
                    r2 = sm.tile([P, 1], f32, tag="r2", name="r2")
                    nc.vector.tensor_tensor(m1[:], att[:], cb["wab"][:], ALU.mult)
                    nc.vector.reduce_sum(r1[:], m1[:], axis=X)
                    nc.vector.tensor_tensor(m1[:], xr[:], cb["wbb"][:], ALU.mult)
                    nc.vector.reduce_sum(r2[:], m1[:], axis=X)
                    nc.vector.tensor_add(r1[:], r1[:], r2[:])
                    beta = sm.tile([P, 1], f32, tag="beta", name="beta")
                    nc.scalar.activation(beta[:], r1[:], AF.Sigmoid)
                    nc.vector.tensor_sub(m1[:], xr[:], att[:])
                    nc.vector.tensor_scalar(m1[:], m1[:], beta[:, :1], None,
                                            ALU.mult)
                    cte = ep.tile([P, P], f32, tag="cte", name="cte")
                    nc.vector.tensor_add(cte[:], att[:], m1[:])
                    nc.scalar.activation(cte[:], cte[:], AF.Gelu)
                    nc.vector.tensor_add(cte[:], cte[:], h_own[:, t, :])
                    hmid = ep.tile([P, P], f32, tag="hmid", name="hmid")
                    _ln(nc, sm, hmid, cte, cb["g1b"], cb["b1b"])
                    hmb = ep.tile([P, P], bf16, tag="hmb", name="hmb")
                    nc.vector.tensor_copy(hmb[:], hmid[:])
                    ptr = bigb("ptrA")
                    nc.tensor.transpose(ptr[:, :P], hmb[:], ident_bf[:])
                    hmT = ep.tile([P, P], bf16, tag="hmT", name="hmT")
                    nc.any.tensor_copy(hmT[:], ptr[:, :P])
                    p1 = big("p1")
                    nc.tensor.matmul(p1[:, :256], lhsT=hmT[:], rhs=W1_t[:],
                                     start=True, stop=True)
                    h2 = ep.tile([P, 256], f32, tag="h2", name="h2")
                    nc.vector.tensor_add(h2[:], p1[:, :256], bf1b_t[:])
                    h2b = ep.tile([P, 256], bf16, tag="h2b", name="h2b")
                    nc.scalar.activation(h2b[:], h2[:], AF.Gelu)
                    h2T = ep.tile([P, 2, P], bf16, tag="h2T", name="h2T")
                    for q in range(2):
                        ptq = bigb("ptrB")
                        nc.tensor.transpose(ptq[:, :P], h2b[:, q * P:(q + 1) * P],
                                            ident_bf[:])
                        nc.any.tensor_copy(h2T[:, q, :], ptq[:, :P])
                    p2 = pair("p2")
                    nc.tensor.matmul(p2[:, 0, :P], lhsT=h2T[:, 0, :], rhs=W2a_t[:],
                                     start=True, stop=False)
                    nc.tensor.matmul(p2[:, 0, :P], lhsT=h2T[:, 1, :], rhs=W2b_t[:],
                                     start=False, stop=True)
                    ff = ep.tile([P, P], f32, tag="ff", name="ff")
                    nc.vector.tensor_add(ff[:], p2[:, 0, :P], cb["bf2b"][:])
                    nc.vector.tensor_add(ff[:], ff[:], hmid[:])
                    hnew = ep.tile([P, P], f32, tag="hnew", name="hnew")
                    _ln(nc, sm, hnew, ff, cb["g2b"], cb["b2b"])
                    nc.vector.tensor_copy(h_own[:, t, :], hnew[:])
                    if debug and li == 0:
                        nc.sync.dma_start(DBG_H[t * P:(t + 1) * P, :], hnew[:])
                    hnb = ep.tile([P, P], bf16, tag="hnb", name="hnb")
                    nc.vector.tensor_copy(hnb[:], hnew[:])
                    nc.sync.dma_start(ag_in[t * P:(t + 1) * P, :], hnb[:])
                    ptn = bigb("ptrC")
                    nc.tensor.transpose(ptn[:, :P], hnb[:], ident_bf[:])
                    nc.any.tensor_copy(hTo[:, t * P:(t + 1) * P], ptn[:, :P])

                ag_out = dr.tile([NPAD, P], bf16, addr_space="Shared",
                                 tag="ag_out", name="ag_out")
                nc.gpsimd.collective_compute(
                    "AllGather", ALU.bypass,
                    replica_groups=[list(range(NCORES))],
                    ins=[ag_in[:].opt()], outs=[ag_out[:].opt()])
                nc.sync.dma_start_transpose(hT[:], ag_out[:])

            # ---------------- pooling ----------------
            wg1 = sm.tile([P, P], bf16, tag="wg1", name="wg1")
            bg1b_t = sm.tile([P, P], f32, tag="bg1b", name="bg1b_t")
            wg2b_t = sm.tile([P, P], f32, tag="wg2b", name="wg2b_t")
            nc.sync.dma_start(wg1[:], ti["Wg1"][:])
            nc.sync.dma_start(bg1b_t[:], ti["bg1b"][:])
            nc.sync.dma_start(wg2b_t[:], ti["wg2b"][:])
            ppool = pair("ppool")
            for t in range(NBLK):
                pg = big("pg")
                nc.tensor.matmul(pg[:, :P], lhsT=hTo[:, t * P:(t + 1) * P],
                                 rhs=wg1[:], start=True, stop=True)
                g1 = ep.tile([P, P], f32, tag="g1", name="g1")
                nc.vector.tensor_add(g1[:], pg[:, :P], bg1b_t[:])
                nc.scalar.activation(g1[:], g1[:], AF.Relu)
                nc.vector.tensor_tensor(g1[:], g1[:], wg2b_t[:], ALU.mult)
                gate = sm.tile([P, 1], f32, tag="gate", name="gate")
                nc.vector.reduce_sum(gate[:], g1[:], axis=X)
                eg = sm.tile([P, 1], f32, tag="eg", name="eg")
                nc.scalar.activation(eg[:], gate[:], AF.Exp, bias=float(bg2))
                nc.vector.tensor_tensor(eg[:], eg[:], padm_t[:, t:t + 1], ALU.mult)
                rhs = ep.tile([P, 132], bf16, tag="prhs", name="prhs")
                nc.vector.tensor_scalar(rhs[:, :P], h_own[:, t, :], eg[:, :1],
                                        None, ALU.mult)
                nc.vector.tensor_copy(rhs[:, P:P + 1], eg[:])
                nc.tensor.matmul(ppool[:NG, 0, :], lhsT=ohg_t[:, t, :],
                                 rhs=rhs[:], start=(t == 0), stop=(t == NBLK - 1))
            psum_sb = sm.tile([NG, 132], f32, tag="psum_sb", name="psum_sb")
            nc.vector.tensor_copy(psum_sb[:], ppool[:NG, 0, :])
            nc.sync.dma_start(ar_in[:], psum_sb[:])
            nc.gpsimd.collective_compute(
                "AllReduce", ALU.add,
                replica_groups=[list(range(NCORES))],
                ins=[ar_in[:].opt()], outs=[ar_out[:].opt()])
            pooled = sm.tile([NG, 132], f32, tag="pooled", name="pooled")
            nc.sync.dma_start(pooled[:], ar_out[:])
            rsp = sm.tile([NG, 1], f32, tag="rsp", name="rsp")
            nc.vector.tensor_scalar(rsp[:], pooled[:, P:P + 1], S_EPS, None,
                                    ALU.add)
            nc.vector.reciprocal(rsp[:], rsp[:])
            pb = sm.tile([NG, P], bf16, tag="pb", name="pbt")
            nc.vector.tensor_scalar(pb[:], pooled[:, :P], rsp[:, :1], None,
                                    ALU.mult)
            ptp = bigb("ptp")
            nc.tensor.transpose(ptp[:, :NG], pb[:], ident_bf[:NG, :NG])
            pT = sm.tile([P, NG], bf16, tag="pT", name="pT")
            nc.any.tensor_copy(pT[:], ptp[:, :NG])
            wo1 = sm.tile([P, P], bf16, tag="wo1", name="wo1")
            bo1c_t = sm.tile([P, 1], f32, tag="bo1c", name="bo1c_t")
            wo2 = sm.tile([P, 2], bf16, tag="wo2", name="wo2")
            bo2c_t = sm.tile([2, 1], f32, tag="bo2c", name="bo2c_t")
            nc.sync.dma_start(wo1[:], ti["Wo1"][:])
            nc.sync.dma_start(bo1c_t[:], ti["bo1c"][:])
            nc.sync.dma_start(wo2[:], ti["Wo2"][:])
            nc.sync.dma_start(bo2c_t[:], ti["bo2c"][:])
            po1 = pair("po1")
            nc.tensor.matmul(po1[:, 0, :NG], lhsT=wo1[:], rhs=pT[:],
                             start=True, stop=True)
            o1 = sm.tile([P, NG], bf16, tag="o1", name="o1")
            nc.scalar.activation(o1[:], po1[:, 0, :NG], AF.Gelu,
                                 bias=bo1c_t[:, :1])
            po2 = pair("po2")
            nc.tensor.matmul(po2[:2, 0, :NG], lhsT=wo2[:], rhs=o1[:],
                             start=True, stop=True)
            ofin = sm.tile([2, NG], f32, tag="ofin", name="ofin")
            nc.scalar.activation(ofin[:], po2[:2, 0, :NG], AF.Identity,
                                 bias=bo2c_t[:, :1])
            nc.sync.dma_start(OUT[:].rearrange("n c -> c n"), ofin[:])
    nc.finalize()
    return nc


_CACHE = {}


def kernel(x, edge_index, batch, params, _layers=LAYERS, _debug=False):
    in_maps, blk_sched, nchunk, bg2 = _preprocess(x, edge_index, batch, params)
    key = (tuple(blk_sched), nchunk, bg2, _layers, _debug)
    if key not in _CACHE:
        _CACHE[key] = _build(blk_sched, nchunk, bg2, layers=_layers,
                             debug=_debug)
    nc = _CACHE[key]
    res = run_bass_kernel_spmd(nc, in_maps, core_ids=list(range(NCORES)))
    if _debug:
        return res
    return np.asarray(res.results[0]["OUT"])
